# revision 35
# baseline (speedup 1.0000x reference)
"""Trainium2 Bass kernel: Tacotron-style location-sensitive attention step.

Sharding (8 NeuronCores, SPMD):
  - Batch dim sharded for everything per-example (enc_seq, proc_mem,
    conv windows, softmax, context): 16 examples per core.
  - LSTM cell H-sharded: core j computes hT rows [128j, 128j+128) for the
    FULL batch from 1/8 of W_ih/W_hh, then contributes a partial
    qry2 = h @ W_q.T which is combined with a ReduceScatter (each core
    receives the summed qry2 rows for exactly its 16 examples).

All heavy operands are pre-cast to bf16 AND pre-transposed/pre-windowed on
the host so that every device DMA is a plain (no-cast) HWDGE transfer and
the tensor engine never transposes weights:
  - wt: LSTM weights in [k-part, gate, k-chunk, hl] layout
  - win2: conv input windows (62 taps, zero-padded), 2 examples packed
    per 128-partition tile
  - wcomb: W_loc folded into the conv kernel (the F=32 conv-channel dim
    is contracted away on the host), so location features come out of a
    single K=64 matmul per (example, half)
  - enc: row-permuted so the context matmul s-chunks line up with the
    softmax transpose layout while DMAs stay contiguous per partition
  - procT: per-example transposed to [A, S] so qry2 can be added as a
    per-partition ACT bias inside the tanh

Compute: bf16 operands for matmuls/elementwise, f32 PSUM + softmax.

kernel(**inputs) takes FULL numpy inputs and returns FULL [128, 512] f32
context.
"""

import sys

sys.path.insert(0, "/opt/trn_rl_repo")

import numpy as np
import ml_dtypes

import concourse.bass as bass
import concourse.mybir as mybir
from concourse import bacc
from concourse.bass_utils import run_bass_kernel_spmd
from concourse.masks import make_identity
from concourse.bass import _add_dep_helper
from concourse.tile import TileContext

F32 = mybir.dt.float32
BF16 = mybir.dt.bfloat16
FP8 = mybir.dt.float8e4
F8 = ml_dtypes.float8_e4m3
AF = mybir.ActivationFunctionType
BF = ml_dtypes.bfloat16

B, S, E, P, H, A, F, KW = 128, 1024, 512, 256, 1024, 128, 32, 31
NCORES = 8
BL = B // NCORES        # 16 examples per core
HL = H // NCORES        # 128 h rows per core
PE_DIM = P + E          # 768
NKI = PE_DIM // 128     # 6
NKH = H // 128          # 8
NK = NKI + NKH          # 14
NC_S = S // 128         # 8 s-chunks
TAPS = 2 * KW           # 62
ENC_BUFS = 14


def build():
    nc = bacc.Bacc("TRN2", target_bir_lowering=False, debug=False,
                   num_devices=NCORES)

    dp = nc.declare_dram_parameter
    wt_a = dp("wt_a", [128, 2, NK, HL], BF16, isOutput=False)
    wt_b = dp("wt_b", [128, 2, NK, HL], BF16, isOutput=False)
    xT = dp("xT", [128, NKI, B], BF16, isOutput=False)
    ahT = dp("ahT", [128, NKH, B], BF16, isOutput=False)
    acT = dp("acT", [HL, B], BF16, isOutput=False)
    bias = dp("bias", [HL, 4], F32, isOutput=False)
    wqT = dp("wqT", [HL, A], BF16, isOutput=False)
    const_a = dp("const_a", [A, 1], F32, isOutput=False)
    wo_col = dp("wo_col", [A, 1], BF16, isOutput=False)
    wcomb2 = dp("wcomb2", [128, A], BF16, isOutput=False)
    win2 = dp("win2", [BL // 2, 128, S], BF16, isOutput=False)
    procT = dp("procT", [BL, A, S], FP8, isOutput=False)
    enc = dp("enc", [BL, 128, NC_S * E], BF16, isOutput=False)
    out = dp("out", [BL, E], F32, isOutput=True)

    with TileContext(nc) as tc:
        with (
            tc.tile_pool(name="const", bufs=1) as cpool,
            tc.tile_pool(name="win", bufs=2) as winp,
            tc.tile_pool(name="proc", bufs=6) as procp,
            tc.tile_pool(name="vsb", bufs=BL) as vsbp,
            tc.tile_pool(name="enc", bufs=BL) as encp,
            tc.tile_pool(name="ctx", bufs=2) as ctxp,
            tc.tile_pool(name="psA", bufs=3, space="PSUM") as psA,
            tc.tile_pool(name="psS", bufs=1, space="PSUM") as psS,
            tc.tile_pool(name="psT", bufs=1, space="PSUM") as psT,
            tc.tile_pool(name="psX", bufs=2, space="PSUM") as psX,
            tc.tile_pool(name="dram", bufs=1, space="DRAM") as dpool,
        ):
            ident = cpool.tile([128, 128], F32)
            make_identity(nc, ident[:])

            # PE warm-up: release the HAM clock gate while weights stream in
            dummy = cpool.tile([128, 256], BF16)
            nc.vector.memset(dummy[:], 0.0)
            for w in range(40):
                ps = psA.tile([128, 512], F32, tag="mm")
                nc.tensor.matmul(ps[:, :256], dummy[:, :128], dummy[:],
                                 start=True, stop=True)

            # ---------------- DMAs: LSTM critical path first ----------------
            # All bulk streaming DMA rides the sync ring (the Sync engine
            # issues nothing else, so HWDGE ring backpressure is harmless);
            # the scalar ring carries only small, critical transfers so the
            # ACT engine's instruction queue stays free for activations.
            wtp_cm = tc.tile_pool(name="wt", bufs=1)
            wtp = wtp_cm.__enter__()
            wta_sb = wtp.tile([128, 2, NK, HL], BF16)
            nc.sync.dma_start(wta_sb[:], wt_a[:])
            wtb_sb = wtp.tile([128, 2, NK, HL], BF16)
            nc.scalar.dma_start(wtb_sb[:], wt_b[:])
            xT_sb = wtp.tile([128, NKI, B], BF16)
            nc.scalar.dma_start(xT_sb[:], xT[:])
            ahT_sb = wtp.tile([128, NKH, B], BF16)
            nc.scalar.dma_start(ahT_sb[:], ahT[:])
            acT_sb = wtp.tile([HL, B], BF16)
            nc.scalar.dma_start(acT_sb[:], acT[:])
            bias_sb = wtp.tile([HL, 4], F32)
            nc.scalar.dma_start(bias_sb[:], bias[:])
            wqT_sb = cpool.tile([HL, A], BF16)
            nc.scalar.dma_start(wqT_sb[:], wqT[:])
            wcomb_sb = cpool.tile([128, A], BF16)
            nc.scalar.dma_start(wcomb_sb[:], wcomb2[:])
            wo_sb = cpool.tile([A, 1], BF16)
            nc.scalar.dma_start(wo_sb[:], wo_col[:])
            ca_sb = cpool.tile([A, 1], F32)
            nc.scalar.dma_start(ca_sb[:], const_a[:])

            # enc stream part 1 on the sync ring, behind wt_a; the rest is
            # issued after the collective (DMA lull so the ReduceScatter's
            # ring steps aren't starved of SDMA engine time)
            ENC_EARLY = 6
            enc_sb = []
            for b in range(ENC_EARLY):
                et = encp.tile([128, NC_S, E], BF16, tag="enc")
                nc.sync.dma_start(
                    et[:], enc[b].rearrange("p (r e) -> p r e", r=NC_S))
                enc_sb.append(et)

            # ---------------- LSTM gates (H-shard, full batch) ----------------
            gate_sb = []
            for g in range(4):
                wsb = wta_sb if g < 2 else wtb_sb
                gg = g % 2
                ps = psA.tile([128, 512], F32, tag="mm")
                for k in range(NKI):
                    nc.tensor.matmul(ps[:, :B], wsb[:, gg, k, :], xT_sb[:, k, :],
                                     start=(k == 0), stop=False)
                for k in range(NKH):
                    nc.tensor.matmul(ps[:, :B], wsb[:, gg, NKI + k, :],
                                     ahT_sb[:, k, :],
                                     start=False, stop=(k == NKH - 1))
                sb = wtp.tile([HL, B], BF16, tag=f"gate{g}")
                fn = AF.Tanh if g == 2 else AF.Sigmoid
                nc.scalar.activation(sb[:], ps[:, :B], fn,
                                     bias=bias_sb[:, g:g + 1])
                gate_sb.append(sb)

            cT = wtp.tile([HL, B], BF16)
            nc.vector.tensor_mul(cT[:], gate_sb[1][:], acT_sb[:])
            tg = wtp.tile([HL, B], BF16)
            nc.vector.tensor_mul(tg[:], gate_sb[0][:], gate_sb[2][:])
            nc.vector.tensor_add(cT[:], cT[:], tg[:])
            nc.scalar.activation(tg[:], cT[:], AF.Tanh)
            hT_sh = wtp.tile([HL, B], BF16)
            nc.vector.tensor_mul(hT_sh[:], gate_sb[3][:], tg[:])

            # partial qry2 for the full batch: [B, A]
            ps_q = psA.tile([128, 512], F32, tag="mm")
            nc.tensor.matmul(ps_q[:, :A], hT_sh[:], wqT_sb[:],
                             start=True, stop=True)
            q_sb = wtp.tile([B, A], F32)
            nc.vector.tensor_copy(q_sb[:], ps_q[:, :A])

            qin = dpool.tile([B, A], F32)
            nc.gpsimd.dma_start(qin[:], q_sb[:])
            wtp_cm.__exit__(None, None, None)

            qout = dpool.tile([BL, A], F32)
            nc.gpsimd.collective_compute(
                "ReduceScatter",
                mybir.AluOpType.add,
                replica_groups=[list(range(NCORES))],
                ins=[qin[:].opt()],
                outs=[qout[:].opt()],
            )
            qg_sb = cpool.tile([BL, A], F32)
            qg_dma = nc.gpsimd.dma_start(qg_sb[:], qout[:])

            # ---- streaming DMAs around the collective ----
            # first 5 window pairs + 10 proc tiles on the scalar ring (after
            # the gate activations in the ACT queue, so they don't delay
            # them); the rest held on the sync ring behind a dep on the
            # collective result, together with the late enc tiles.
            WQ_EARLY, PB_EARLY = 5, 10
            win_sb = []
            proc_sb = []
            for q in range(WQ_EARLY):
                wq_t = winp.tile([128, S], BF16, tag="win")
                nc.scalar.dma_start(wq_t[:], win2[q])
                win_sb.append(wq_t)
                for e2 in range(2):
                    pt = procp.tile([A, S], FP8, tag="proc")
                    nc.scalar.dma_start(pt[:], procT[2 * q + e2])
                    proc_sb.append(pt)
            held_first = None
            for q in range(WQ_EARLY, BL // 2):
                wq_t = winp.tile([128, S], BF16, tag="win")
                dma = nc.sync.dma_start(wq_t[:], win2[q])
                if held_first is None:
                    held_first = dma
                    _add_dep_helper(dma.ins, qg_dma.ins, sync=True,
                                    reason="DMA lull during ReduceScatter")
                win_sb.append(wq_t)
                for e2 in range(2):
                    pt = procp.tile([A, S], FP8, tag="proc")
                    nc.sync.dma_start(pt[:], procT[2 * q + e2])
                    proc_sb.append(pt)
            for b in range(ENC_EARLY, BL):
                et = encp.tile([128, NC_S, E], BF16, tag="enc")
                nc.sync.dma_start(
                    et[:], enc[b].rearrange("p (r e) -> p r e", r=NC_S))
                enc_sb.append(et)

            # ---------------- location features ----------------
            # v[b] = (W_loc-folded conv)(windows) in [A, S] layout, + proc.T
            v_sb = []
            for q in range(BL // 2):
                wq_t = win_sb[q]
                for e2 in range(2):
                    b = 2 * q + e2
                    pt = proc_sb[b]
                    vt = vsbp.tile([A, S], BF16, tag="v")
                    base = 64 * e2
                    for c in range(2):
                        ps_v = psA.tile([128, 512], F32, tag="mm")
                        nc.tensor.matmul(
                            ps_v,
                            wcomb_sb[base:base + 64, :],
                            wq_t[base:base + 64, c * 512:(c + 1) * 512],
                            start=True, stop=True)
                        nc.vector.tensor_add(
                            vt[:, c * 512:(c + 1) * 512], ps_v,
                            pt[:, c * 512:(c + 1) * 512])
                    v_sb.append(vt)

            # ---------------- qry2 (post-RS) ----------------
            ps_t = psS.tile([128, 128], F32, tag="s")
            nc.tensor.transpose(ps_t[:, :BL], qg_sb[:], ident[:BL, :BL])
            qry2T = cpool.tile([A, BL], F32)
            nc.scalar.activation(qry2T[:], ps_t[:, :BL], AF.Identity,
                                 bias=ca_sb[:])

            # ---------------- scores ----------------
            # t = tanh(v + qry2[b]) ; scoresT[s-chunk, (c, b)] = t.T @ w_out
            # tanh in half-tiles so the scores matmuls pipeline behind it
            scT_ps = psS.tile([128, 128], F32, tag="s")
            for b in range(BL):
                for h in range(2):
                    nc.scalar.activation(
                        v_sb[b][:, h * 512:(h + 1) * 512],
                        v_sb[b][:, h * 512:(h + 1) * 512],
                        AF.Tanh, bias=qry2T[:, b:b + 1])
                    for c in range(4 * h, 4 * h + 4):
                        nc.tensor.matmul(
                            scT_ps[:, c * BL + b:c * BL + b + 1],
                            v_sb[b][:, c * 128:(c + 1) * 128],
                            wo_sb[:],
                            start=True, stop=True)
            scT_sb = cpool.tile([128, 128], F32)
            nc.vector.tensor_copy(scT_sb[:], scT_ps[:])

            # ---------------- softmax over S, all 16 examples ----------------
            # (scores are bounded by ||w_out||_1 ~ 5, so exp needs no
            # max-subtraction in f32)
            sc_ps = psT.tile([BL, S], F32, tag="t")
            for c in range(NC_S):
                nc.tensor.transpose(sc_ps[:, c * 128:(c + 1) * 128],
                                    scT_sb[:, c * BL:(c + 1) * BL],
                                    ident[:])
            sums = cpool.tile([BL, 1], F32)
            wts = cpool.tile([BL, S], F32)
            nc.scalar.activation(wts[:], sc_ps[:], AF.Exp,
                                 accum_out=sums[:])
            rs = cpool.tile([BL, 1], F32)
            nc.vector.reciprocal(rs[:], sums[:])
            nc.vector.tensor_scalar_mul(wts[:], wts[:], rs[:])

            wtsT = cpool.tile([128, NC_S, BL], BF16)
            for c in range(NC_S):
                ps_w = psS.tile([128, 128], F32, tag="s")
                nc.tensor.transpose(ps_w[:, :BL],
                                    wts[:, c * 128:(c + 1) * 128],
                                    ident[:BL, :BL])
                nc.vector.tensor_copy(wtsT[:, c, :], ps_w[:, :BL])

            # ---------------- context ----------------
            for grp in range(BL // 4):
                psx = psX.tile([128, 512], F32, tag="x")
                nc.vector.memset(psx[:], 0.0)
                for i in range(4):
                    b = 4 * grp + i
                    for c in range(NC_S):
                        nc.tensor.matmul(
                            psx[32 * i:32 * i + 1, :],
                            wtsT[:, c, b:b + 1],
                            enc_sb[b][:, c, :],
                            start=(c == 0), stop=(c == NC_S - 1),
                            tile_position=(0, 32 * i))
                ctx_sb = ctxp.tile([128, 512], F32, tag="ctx")
                nc.vector.tensor_copy(ctx_sb[:], psx[:])
                nc.sync.dma_start(out[4 * grp:4 * grp + 4, :],
                                  ctx_sb[0:128:32, :])

    nc.compile()
    return nc


_NC_CACHE = None


def _get_nc():
    global _NC_CACHE
    if _NC_CACHE is None:
        _NC_CACHE = build()
    return _NC_CACHE


def shard_inputs(prenet, prev_context, att_h, att_c, prev_weights, cum_weights,
                 enc_seq, proc_mem, mask, W_ih, W_hh, b_ih, b_hh, conv_w,
                 conv_b, W_loc, b_loc, W_q, b_q, W_out, b_out, **_unused):
    f32 = np.float32
    c = np.ascontiguousarray

    W_ih4 = np.asarray(W_ih, f32).reshape(4, H, PE_DIM)
    W_hh4 = np.asarray(W_hh, f32).reshape(4, H, H)
    bias4 = (np.asarray(b_ih, f32) + np.asarray(b_hh, f32)).reshape(4, H)

    x = np.concatenate([np.asarray(prenet, f32),
                        np.asarray(prev_context, f32)], axis=1)  # [B, 768]
    xT_h = c(x.T.reshape(NKI, 128, B).transpose(1, 0, 2).astype(BF))
    ahT_h = c(np.asarray(att_h, f32).T.reshape(NKH, 128, B)
              .transpose(1, 0, 2).astype(BF))

    # W_loc folded into the conv kernel: wcomb[(c,k), a]
    cw = np.asarray(conv_w, f32).reshape(F, TAPS)          # [F, 62]
    wcomb = (np.asarray(W_loc, f32) @ cw).T                # [62, A]
    wcomb2_h = np.zeros((128, A), f32)
    wcomb2_h[0:TAPS] = wcomb
    wcomb2_h[64:64 + TAPS] = wcomb
    wcomb2_h = c(wcomb2_h.astype(BF))

    # constant additive term for the tanh argument (per A)
    const = (np.asarray(b_q, f32) + np.asarray(b_loc, f32)
             + np.asarray(W_loc, f32) @ np.asarray(conv_b, f32))  # [A]
    const_h = c(const.reshape(A, 1))
    wo_h = c(np.asarray(W_out, f32).reshape(A, 1).astype(BF))

    # conv windows (padded), per example: [62, S]
    cum = np.asarray(cum_weights, f32)
    prv = np.asarray(prev_weights, f32)
    padded = np.zeros((B, 2, KW // 2 + S + KW // 2 + 1), f32)
    padded[:, 0, KW // 2:KW // 2 + S] = cum
    padded[:, 1, KW // 2:KW // 2 + S] = prv
    sw = np.lib.stride_tricks.sliding_window_view(
        padded, S, axis=2)                                  # [B, 2, KW+1, S]
    win = sw[:, :, :KW, :].reshape(B, TAPS, S)              # [B, 62, S]

    enc_bf = np.asarray(enc_seq, f32).reshape(B, NC_S, 128, E) \
        .transpose(0, 2, 1, 3).reshape(B, 128, NC_S * E).astype(BF)
    procT_bf = np.asarray(proc_mem, f32).transpose(0, 2, 1).astype(BF)

    in_maps = []
    for j in range(NCORES):
        bj = slice(BL * j, BL * (j + 1))
        hj = slice(HL * j, HL * (j + 1))

        wt = np.concatenate(
            [W_ih4[:, hj].reshape(4, HL, NKI, 128),
             W_hh4[:, hj].reshape(4, HL, NKH, 128)], axis=2)  # [4, HL, 14, 128]
        wt = wt.transpose(3, 0, 2, 1).astype(BF)              # [128, 4, 14, HL]

        win_j = win[bj]                                       # [16, 62, S]
        win2_h = np.zeros((BL // 2, 128, S), f32)
        win2_h[:, 0:TAPS] = win_j[0::2]
        win2_h[:, 64:64 + TAPS] = win_j[1::2]

        in_maps.append({
            "wt_a": c(wt[:, 0:2]),
            "wt_b": c(wt[:, 2:4]),
            "xT": xT_h,
            "ahT": ahT_h,
            "acT": c(np.asarray(att_c, f32)[:, hj].T.astype(BF)),
            "bias": c(bias4[:, hj].T),
            "wqT": c(np.asarray(W_q, f32)[:, hj].T.astype(BF)),
            "const_a": const_h,
            "wo_col": wo_h,
            "wcomb2": wcomb2_h,
            "win2": c(win2_h.astype(BF)),
            "procT": c(procT_bf[bj].astype(F8)),
            "enc": c(enc_bf[bj]),
        })
    return in_maps


def kernel(**inputs):
    assert not np.any(np.asarray(inputs["mask"])), \
        "kernel assumes mask == 0 (softmax-shift support not implemented)"
    nc = _get_nc()
    in_maps = shard_inputs(**inputs)
    res = run_bass_kernel_spmd(nc, in_maps, core_ids=list(range(NCORES)))
    return np.concatenate([res.results[j]["out"] for j in range(NCORES)],
                          axis=0)


if __name__ == "__main__":
    print("building...")
    _get_nc()
    print("built ok")
